# revision 46
# baseline (speedup 1.0000x reference)
"""Trainium2 Bass kernel for 16-head causal RoPE attention (B=1, L=4096, D=1024).

Distribution: tensor-parallel over heads — each of the 8 cores owns 2 heads
(128 q/k/v dims) and computes a partial output projection; the host sums the
8 partial [1024, 4096] outputs (bf16) and transposes back to [1, 4096, 1024].

v2 design (~272us vs ~353us v1 baseline):
  - all-bf16 matmul datapath (x, w, q/k/v, rope tables, attn weights, y out)
  - phase1 (projection+RoPE) for chunk n+1 and the norm+wo of chunk n-1 are
    interleaved into attention(n)'s emission so the PE queue stays dense and
    the HAM clock-gate holds 2.4GHz (idle gaps re-throttle it to 1.2GHz)
  - causal trimming of the diagonal 512x512 block (st/exp/av/mask operate on
    the needed column suffix only)
  - softmax denominators: SBUF->SBUF DMA pack to [16,64], one cheap DVE
    reciprocal (8 cyc/elem scales with free size), DMA unpack, broadcast to
    128 partitions via a K=2 PE matmul; the PE-dependent half (norm_b) is
    deferred into the next chunk so the PE never waits on the recip chain;
    the unnormalized attn output is copied to SBUF at flush time to release
    the av PSUM slots (avoids a pool-slot deadlock with the deferral)
  - per-chunk qro/kro/vno/outT tiles to avoid false cross-phase deps
  - prologue DMA descriptor issues spread over sync/scalar/gpsimd queues
    (issue is ~600ns serial per queue); bulk x/table prefetch staggered one
    chunk ahead of use; y drains via SBUF bf16 copies (DMA cannot read PSUM)
"""

import numpy as np

N_HEAD = 16
HEAD_DIM = 64
HIDDEN = 1024
N_CORES = 8
ROPE_BASE = 10000.0

_CACHE = {}


def _build(L):
    import concourse.bass as bass
    import concourse.tile as tile
    import concourse.mybir as mybir
    from concourse import bacc
    from concourse.masks import make_identity

    F32 = mybir.dt.float32
    BF16 = mybir.dt.bfloat16
    Exp = mybir.ActivationFunctionType.Exp

    LC = L // 512          # number of 512-wide q chunks
    KVC = L // 128         # number of 128-wide kv chunks
    HC = HIDDEN // 128     # hidden contraction chunks

    nc = bacc.Bacc("TRN2", target_bir_lowering=False, debug=False,
                   num_devices=N_CORES)

    xT_d = nc.dram_tensor("xT", [HIDDEN, L], BF16, kind="ExternalInput")
    wqT_d = nc.dram_tensor("wqT", [HIDDEN, 128], BF16, kind="ExternalInput")
    wkT_d = nc.dram_tensor("wkT", [HIDDEN, 128], BF16, kind="ExternalInput")
    wvT_d = nc.dram_tensor("wvT", [HIDDEN, 128], BF16, kind="ExternalInput")
    woT_d = nc.dram_tensor("woT", [128, HIDDEN], BF16, kind="ExternalInput")
    cosT_d = nc.dram_tensor("cosT", [128, L], BF16, kind="ExternalInput")
    sinT_d = nc.dram_tensor("sinT", [128, L], BF16, kind="ExternalInput")
    tri_d = nc.dram_tensor("tri", [128, 128], BF16, kind="ExternalInput")
    pmat_d = nc.dram_tensor("pmat", [128, 128], BF16, kind="ExternalInput")
    bsel_d = nc.dram_tensor("bsel", [2, 128], BF16, kind="ExternalInput")
    yT_d = nc.dram_tensor("yT", [HIDDEN, L], BF16, kind="ExternalOutput")

    with tile.TileContext(nc) as tc:
        with tc.tile_pool(name="big", bufs=1) as big, \
             tc.tile_pool(name="w_p", bufs=1) as w_p, \
             tc.tile_pool(name="sm2", bufs=3) as sm2, \
             tc.tile_pool(name="att_p", bufs=6) as att_p, \
             tc.tile_pool(name="y_p", bufs=6) as y_p, \
             tc.tile_pool(name="ps_acc", bufs=2, space="PSUM") as ps_acc, \
             tc.tile_pool(name="ps_st", bufs=2, space="PSUM") as ps_st, \
             tc.tile_pool(name="ps_av", bufs=2, space="PSUM") as ps_av:

            # ---- persistent SBUF tensors ----
            wq_sb = w_p.tile([128, HC, 128], BF16, tag="wq")
            wk_sb = w_p.tile([128, HC, 128], BF16, tag="wk")
            wv_sb = w_p.tile([128, HC, 128], BF16, tag="wv")
            wo_sb = w_p.tile([128, HIDDEN], BF16, tag="wo")
            tri_sb = w_p.tile([128, 128], BF16, tag="tri")
            pmat_sb = w_p.tile([128, 128], BF16, tag="pmat")
            bsel_sb = w_p.tile([2, 128], BF16, tag="bsel")
            ident = w_p.tile([128, 128], BF16, tag="ident")
            cs_sb = w_p.tile([128, L], BF16, tag="cs")
            sn_sb = w_p.tile([128, L], BF16, tag="sn")
            xt_sb = w_p.tile([128, HC, L], BF16, tag="xt")

            qro = [big.tile([128, 512], BF16, tag=f"qro{n}", name=f"qro{n}")
                   for n in range(LC)]
            kro = [big.tile([128, 512], BF16, tag=f"kro{n}", name=f"kro{n}")
                   for n in range(LC)]
            outT = [big.tile([128, 512], BF16, tag=f"out{n}", name=f"out{n}")
                    for n in range(LC)]
            vno = [big.tile([128, 130], BF16, tag=f"vno{k}", name=f"vno{k}")
                   for k in range(KVC)]

            # ---- prologue ----
            # HAM warmup: the PE clock-gate needs ~3.4us of sustained
            # activity to unthrottle 1.2->2.4GHz; run dummy matmuls on a
            # memset tile while the first weight/x DMAs are in flight
            dmy = w_p.tile([128, 256], BF16, tag="dmy")
            nc.gpsimd.memset(dmy, 1.0)
            for _ in range(12):
                dps = ps_acc.tile([128, 256], F32, tag="acc")
                nc.tensor.matmul(dps, dmy[:, 0:128], dmy)
            # critical DMA issues spread over 3 engine
            # queues (descriptor issue is ~600ns serial per engine) ----
            for quad in range(4):
                ks = slice(quad * 2 * 128, (quad + 1) * 2 * 128)
                kt_ = slice(quad * 2, (quad + 1) * 2)
                nc.sync.dma_start(
                    out=wq_sb[:, kt_, :],
                    in_=wqT_d.ap()[ks, :].rearrange("(c p) m -> p c m", p=128))
                nc.sync.dma_start(
                    out=xt_sb[:, kt_, 0:512],
                    in_=xT_d.ap()[ks, 0:512]
                        .rearrange("(c p) m -> p c m", p=128))
                nc.gpsimd.dma_start(
                    out=wk_sb[:, kt_, :],
                    in_=wkT_d.ap()[ks, :].rearrange("(c p) m -> p c m", p=128))
                nc.scalar.dma_start(
                    out=wv_sb[:, kt_, :],
                    in_=wvT_d.ap()[ks, :].rearrange("(c p) m -> p c m", p=128))
            nc.scalar.dma_start(out=cs_sb[:, 0:512], in_=cosT_d.ap()[:, 0:512])
            nc.scalar.dma_start(out=sn_sb[:, 0:512], in_=sinT_d.ap()[:, 0:512])
            nc.scalar.dma_start(out=pmat_sb, in_=pmat_d.ap())
            nc.scalar.dma_start(out=tri_sb, in_=tri_d.ap())
            nc.scalar.dma_start(out=bsel_sb, in_=bsel_d.ap())
            # staggered bulk prefetch: chunk n's x (4-way k-split) + rope
            # tables, issued from the gpsimd queue
            def prefetch(n):
                if n >= LC:
                    return
                ns = slice(n * 512, (n + 1) * 512)
                for kh in range(4):
                    nc.gpsimd.dma_start(
                        out=xt_sb[:, 2 * kh:2 * kh + 2, ns],
                        in_=xT_d.ap()[kh * 256:(kh + 1) * 256, ns]
                            .rearrange("(c p) m -> p c m", p=128))
                nc.gpsimd.dma_start(out=cs_sb[:, ns], in_=cosT_d.ap()[:, ns])
                nc.gpsimd.dma_start(out=sn_sb[:, ns], in_=sinT_d.ap()[:, ns])

            # prefetch(1) first so chunk 1's x lands before phase1(1);
            # then ones columns of v tiles (vno[kc] needed from
            # attention(kc//4) onward)
            prefetch(1)
            for kc in range(KVC):
                nc.gpsimd.memset(
                    vno[kc].rearrange("p (g c) -> p g c", c=65)[:, :, 64:65],
                    1.0)
            make_identity(nc, ident)

            # ---- phase1: projections + rope + v transpose for chunk n ----
            def accum(n, w_sb, dst_tag):
                ns = slice(n * 512, (n + 1) * 512)
                ps = ps_acc.tile([128, 512], F32, tag="acc")
                for k in range(HC):
                    nc.tensor.matmul(ps, w_sb[:, k, :], xt_sb[:, k, ns],
                                     start=(k == 0), stop=(k == HC - 1))
                t = sm2.tile([128, 512], BF16, tag=dst_tag)
                nc.vector.tensor_copy(t, ps)
                return ps, t

            def rope(n, t_sb, ro):
                ns = slice(n * 512, (n + 1) * 512)
                sw = ps_acc.tile([128, 512], F32, tag="acc")
                nc.tensor.matmul(sw, pmat_sb, t_sb)
                t1 = sm2.tile([128, 512], BF16, tag="t1")
                t2 = sm2.tile([128, 512], BF16, tag="t2")
                nc.vector.tensor_mul(t1, t_sb, cs_sb[:, ns])
                nc.vector.tensor_mul(t2, sw, sn_sb[:, ns])
                nc.vector.tensor_add(ro, t1, t2)

            def p1_parts(n):
                if n >= LC:
                    return []

                def qpart():
                    _, qt = accum(n, wq_sb, "qt")
                    rope(n, qt, qro[n])

                def kpart():
                    _, kt = accum(n, wk_sb, "kt")
                    rope(n, kt, kro[n])

                def vpart():
                    if n == 0:  # wo needed from the first wo projection on
                        nc.gpsimd.dma_start(out=wo_sb[:, 0:512],
                                            in_=woT_d.ap()[:, 0:512])
                        nc.gpsimd.dma_start(out=wo_sb[:, 512:1024],
                                            in_=woT_d.ap()[:, 512:1024])
                    prefetch(n + 2)
                    _, vt = accum(n, wv_sb, "vt")
                    for j in range(4):
                        kc = n * 4 + j
                        tr = ps_acc.tile([128, 128], BF16, tag="acc")
                        nc.tensor.transpose(tr, vt[:, j * 128:(j + 1) * 128],
                                            ident)
                        nc.vector.tensor_copy(
                            vno[kc]
                                .rearrange("p (g c) -> p g c", c=65)[:, :, 0:64],
                            tr.rearrange("p (g c) -> p g c", c=64))

                return [qpart, kpart, vpart]

            # ---- attention for q chunk qc (list of emission steps) ----
            def att_steps(qc):
                qs = slice(qc * 512, (qc + 1) * 512)
                n_kc = 4 * (qc + 1)
                n_g = n_kc // 2
                state = {}

                def qlo_of(kc):
                    d = kc - 4 * qc
                    return 128 * d if d >= 0 else 0

                def st_exp(h, g):
                    hs = h * 64
                    stp = ps_st.tile([128, 2, 512], F32, tag="st")
                    att = att_p.tile([128, 2, 512], BF16, tag="att")
                    for j in range(2):
                        kc = 2 * g + j
                        qlo = qlo_of(kc)
                        nc.tensor.matmul(
                            stp[:, j, qlo:],
                            kro[kc // 4][hs:hs + 64,
                                         (kc % 4) * 128:(kc % 4 + 1) * 128],
                            qro[qc][hs:hs + 64, qlo:])
                    if 2 * g >= 4 * qc:  # diagonal group: per-j trimmed exp
                        for j in range(2):
                            qlo = qlo_of(2 * g + j)
                            nc.scalar.activation(att[:, j, qlo:],
                                                 stp[:, j, qlo:], Exp)
                            nc.vector.tensor_mul(att[:, j, qlo:qlo + 128],
                                                 att[:, j, qlo:qlo + 128],
                                                 tri_sb)
                    else:
                        nc.scalar.activation(att, stp, Exp)
                    return att

                def av_mms(h, g_prev, att_prev):
                    for j in range(2):
                        kc = 2 * g_prev + j
                        qlo = qlo_of(kc)
                        nc.tensor.matmul(
                            state["avs"][h][:, qlo:],
                            vno[kc][:, h * 65:h * 65 + 65],
                            att_prev[:, j, qlo:],
                            start=(kc == 0), stop=(kc == n_kc - 1))

                steps = []

                def mk_gstep(g):
                    def gstep():
                        if g == 0:
                            state["avs"] = [ps_av.tile([65, 512], F32, tag="av",
                                                        name=f"av{qc}_{i}")
                                            for i in range(2)]
                            state["pending"] = [None, None]
                        for h in range(2):
                            att = st_exp(h, g)
                            if state["pending"][h] is not None:
                                av_mms(h, *state["pending"][h])
                            state["pending"][h] = (g, att)
                    return gstep

                for g in range(n_g):
                    steps.append(mk_gstep(g))

                def flush_norm():
                    avs = state["avs"]
                    # per-head: flush av, then immediately copy its
                    # denominator row (starts the recip chain earlier)
                    den = sm2.tile([1, 1024], F32, tag="den")
                    for h in range(2):
                        av_mms(h, *state["pending"][h])
                        nc.vector.tensor_copy(den[0:1, h * 512:(h + 1) * 512],
                                              avs[h][64:65, :])
                    # unnormalized attention output -> SBUF (releases the av
                    # PSUM slots now; deferred norm_b reads the SBUF copy)
                    av_sb = sm2.tile([128, 512], BF16, tag="avsb",
                                     name=f"avsb{qc}", bufs=2)
                    nc.vector.tensor_copy(av_sb[0:64, :], avs[0][0:64, :])
                    nc.vector.tensor_copy(av_sb[64:128, :], avs[1][0:64, :])
                    state["av_sb"] = av_sb
                    dpk = sm2.tile([16, 64], F32, tag="dpk")
                    nc.gpsimd.dma_start(
                        out=dpk,
                        in_=den[0:1, :].rearrange("o (p c) -> o p c", p=16))
                    rpk = sm2.tile([16, 64], BF16, tag="rpk")
                    with nc.allow_low_precision(reason="bf16 1/denominator"):
                        nc.vector.reciprocal(rpk, dpk)
                    rden = sm2.tile([2, 512], BF16, tag="rden",
                                    name=f"rden{qc}", bufs=2)
                    nc.gpsimd.dma_start(
                        out=rden.rearrange("h (p c) -> h p c", p=8),
                        in_=rpk)
                    state["rden"] = rden

                def norm_b():
                    av_sb = state["av_sb"]
                    bc = ps_acc.tile([128, 512], F32, tag="acc")
                    nc.tensor.matmul(bc, bsel_sb, state["rden"])
                    bcs = sm2.tile([128, 512], F32, tag="bcs")
                    nc.vector.tensor_copy(bcs, bc)
                    nc.vector.tensor_mul(outT[qc], av_sb, bcs)
                steps.append(flush_norm)
                return steps, norm_b

            def wo_part(qc):
                qs = slice(qc * 512, (qc + 1) * 512)
                last = qc == LC - 1

                def run():
                    for e in range(HC):
                        ps_y = ps_acc.tile([128, 512], F32, tag="acc")
                        nc.tensor.matmul(ps_y, wo_sb[:, e * 128:(e + 1) * 128],
                                         outT[qc])
                        y_sb = y_p.tile([128, 512], BF16, tag="y")
                        if last and e % 2 == 0:
                            nc.scalar.copy(y_sb, ps_y)
                        else:
                            nc.vector.tensor_copy(y_sb, ps_y)
                        if last and e % 2 == 1:
                            nc.scalar.dma_start(
                                out=yT_d.ap()[e * 128:(e + 1) * 128, qs],
                                in_=y_sb)
                        else:
                            nc.sync.dma_start(
                                out=yT_d.ap()[e * 128:(e + 1) * 128, qs],
                                in_=y_sb)
                return run

            # ---- emission schedule ----
            # process attention chunks 1..7 then 0 (so the tiny chunk 0
            # forms the tail); phase1(qc) for the next chunk and the
            # previous chunk's norm+wo are interleaved into each attention
            for p in p1_parts(0):
                p()
            order = list(range(LC))
            norm_bs = {}
            for idx, qc in enumerate(order):
                steps, norm_b = att_steps(qc)
                norm_bs[qc] = norm_b
                extras = p1_parts(qc + 1)
                prev = order[idx - 1] if idx >= 1 else None
                merged = []
                ne, ns_ = len(extras), len(steps)
                wo_pos = max(1, (7 * ns_) // 10)
                ei = 0
                for i, s in enumerate(steps):
                    merged.append(s)
                    if prev is not None and i + 1 == wo_pos:
                        merged.append(norm_bs[prev])
                        merged.append(wo_part(prev))
                    want = (i + 1) * ne // ns_
                    while ei < want:
                        merged.append(extras[ei])
                        ei += 1
                while ei < ne:
                    merged.append(extras[ei])
                    ei += 1
                for f in merged:
                    f()
            norm_bs[order[-1]]()
            wo_part(order[-1])()

    nc.compile()
    return nc


def _host_prep(x, wq, wk, wv, wo, L):
    """Build per-core input maps (numpy only)."""
    import ml_dtypes
    BF = ml_dtypes.bfloat16

    x2 = np.ascontiguousarray(x.reshape(L, HIDDEN))
    xT = np.ascontiguousarray(x2.T.astype(BF))

    # rope tables, transposed + duplicated for the two heads on each core
    inv_freq = 1.0 / (ROPE_BASE ** (np.arange(0, HEAD_DIM, 2, dtype=np.float64)
                                    / HEAD_DIM))
    freqs = np.arange(L, dtype=np.float64)[:, None] * inv_freq[None, :]
    emb = np.concatenate([freqs, freqs], axis=-1)          # [L, 64]
    cosT = np.cos(emb).T
    sinT = np.sin(emb).T
    cosT2 = np.ascontiguousarray(np.concatenate([cosT, cosT], axis=0).astype(BF))
    sinT2 = np.ascontiguousarray(np.concatenate([sinT, sinT], axis=0).astype(BF))

    # triangular mask for the 128-wide diagonal kv blocks: keep q >= kv
    kv = np.arange(128)[:, None]
    q = np.arange(128)[None, :]
    tri = np.ascontiguousarray((q >= kv).astype(BF))

    # rotate-half permutation (as matmul lhsT), block-diag for 2 heads
    P = np.zeros((64, 64), np.float32)
    P[np.arange(32) + 32, np.arange(32)] = -1.0
    P[np.arange(32), np.arange(32) + 32] = 1.0
    pmat = np.zeros((128, 128), np.float32)
    pmat[0:64, 0:64] = P
    pmat[64:128, 64:128] = P
    pmat = np.ascontiguousarray(pmat.astype(BF))

    # denominator broadcast selector: row h -> partitions [64h, 64h+64)
    bsel = np.zeros((2, 128), np.float32)
    bsel[0, 0:64] = 1.0
    bsel[1, 64:128] = 1.0
    bsel = np.ascontiguousarray(bsel.astype(BF))

    in_maps = []
    for c in range(N_CORES):
        rows = slice(c * 128, (c + 1) * 128)
        in_maps.append({
            "xT": xT,
            "wqT": np.ascontiguousarray(
                (wq[rows, :].T * np.float32(1.0 / 8.0)).astype(BF)),
            "wkT": np.ascontiguousarray(wk[rows, :].T.astype(BF)),
            "wvT": np.ascontiguousarray(wv[rows, :].T.astype(BF)),
            "woT": np.ascontiguousarray(wo[:, rows].T.astype(BF)),
            "cosT": cosT2,
            "sinT": sinT2,
            "tri": tri,
            "pmat": pmat,
            "bsel": bsel,
        })
    return in_maps


def _ensure_profile_hook():
    """The agent image's antenv lacks axon_hooks; recreate it from the boot
    package so trace=True can capture NTFF profiles."""
    import sys, types
    try:
        from antenv.axon_hooks import get_axon_ntff_profile_hook  # noqa: F401
        return
    except ImportError:
        pass
    try:
        from trn_agent_boot.trn_boot import _ntff_profile_via_ctypes
        hook = _ntff_profile_via_ctypes('/opt/axon/libaxon_pjrt.so')
    except Exception:
        hook = None
    mod = types.ModuleType("antenv.axon_hooks")
    mod.get_axon_ntff_profile_hook = lambda: hook
    mod.set_axon_ntff_profile_hook = lambda h: None
    sys.modules["antenv.axon_hooks"] = mod


def _run(x, wq, wk, wv, wo, trace=False, trace_cores=None):
    from concourse.bass_utils import run_bass_kernel_spmd

    if trace:
        _ensure_profile_hook()

    B, L, D = x.shape
    assert (B, D) == (1, HIDDEN)
    if L not in _CACHE:
        _CACHE[L] = _build(L)
    nc = _CACHE[L]
    in_maps = _host_prep(np.asarray(x, np.float32), wq, wk, wv, wo, L)
    res = run_bass_kernel_spmd(
        nc, in_maps, core_ids=list(range(N_CORES)),
        trace=trace, trace_cores=trace_cores)
    acc = np.zeros((HIDDEN, L), np.float64)
    for r in res.results:
        acc += r["yT"].astype(np.float64)
    y = np.ascontiguousarray(acc.T.astype(np.float32)).reshape(1, L, HIDDEN)
    return y, res


def kernel(x, wq, wk, wv, wo):
    y, _ = _run(np.asarray(x), np.asarray(wq), np.asarray(wk),
                np.asarray(wv), np.asarray(wo))
    return y


# revision 47
# speedup vs baseline: 1.1302x; 1.1302x over previous
"""Trainium2 Bass kernel for 16-head causal RoPE attention (B=1, L=4096, D=1024).

Distribution: tensor-parallel over heads — each of the 8 cores owns 2 heads
(128 q/k/v dims) and computes a partial output projection; the host sums the
8 partial [1024, 4096] outputs (bf16) and transposes back to [1, 4096, 1024].

v2 design (~272us vs ~353us v1 baseline):
  - all-bf16 matmul datapath (x, w, q/k/v, rope tables, attn weights, y out)
  - phase1 (projection+RoPE) for chunk n+1 and the norm+wo of chunk n-1 are
    interleaved into attention(n)'s emission so the PE queue stays dense and
    the HAM clock-gate holds 2.4GHz (idle gaps re-throttle it to 1.2GHz)
  - causal trimming of the diagonal 512x512 block (st/exp/av/mask operate on
    the needed column suffix only)
  - softmax denominators: SBUF->SBUF DMA pack to [16,64], one cheap DVE
    reciprocal (8 cyc/elem scales with free size), DMA unpack, broadcast to
    128 partitions via a K=2 PE matmul; the PE-dependent half (norm_b) is
    deferred into the next chunk so the PE never waits on the recip chain;
    the unnormalized attn output is copied to SBUF at flush time to release
    the av PSUM slots (avoids a pool-slot deadlock with the deferral)
  - per-chunk qro/kro/vno/outT tiles to avoid false cross-phase deps
  - prologue DMA descriptor issues spread over sync/scalar/gpsimd queues
    (issue is ~600ns serial per queue); bulk x/table prefetch staggered one
    chunk ahead of use; y drains via SBUF bf16 copies (DMA cannot read PSUM)
"""

import numpy as np

N_HEAD = 16
HEAD_DIM = 64
HIDDEN = 1024
N_CORES = 8
ROPE_BASE = 10000.0

_CACHE = {}


def _build(L):
    import concourse.bass as bass
    import concourse.tile as tile
    import concourse.mybir as mybir
    from concourse import bacc
    from concourse.masks import make_identity

    F32 = mybir.dt.float32
    BF16 = mybir.dt.bfloat16
    Exp = mybir.ActivationFunctionType.Exp

    LC = L // 512          # number of 512-wide q chunks
    KVC = L // 128         # number of 128-wide kv chunks
    HC = HIDDEN // 128     # hidden contraction chunks

    nc = bacc.Bacc("TRN2", target_bir_lowering=False, debug=False,
                   num_devices=N_CORES)

    xT_d = nc.dram_tensor("xT", [HIDDEN, L], BF16, kind="ExternalInput")
    wqT_d = nc.dram_tensor("wqT", [HIDDEN, 128], BF16, kind="ExternalInput")
    wkT_d = nc.dram_tensor("wkT", [HIDDEN, 128], BF16, kind="ExternalInput")
    wvT_d = nc.dram_tensor("wvT", [HIDDEN, 128], BF16, kind="ExternalInput")
    woT_d = nc.dram_tensor("woT", [128, HIDDEN], BF16, kind="ExternalInput")
    cosT_d = nc.dram_tensor("cosT", [128, L], BF16, kind="ExternalInput")
    sinT_d = nc.dram_tensor("sinT", [128, L], BF16, kind="ExternalInput")
    tri_d = nc.dram_tensor("tri", [128, 128], BF16, kind="ExternalInput")
    pmat_d = nc.dram_tensor("pmat", [128, 128], BF16, kind="ExternalInput")
    bsel_d = nc.dram_tensor("bsel", [2, 128], BF16, kind="ExternalInput")
    yT_d = nc.dram_tensor("yT", [HIDDEN, L], BF16, kind="ExternalOutput")

    with tile.TileContext(nc) as tc:
        with tc.tile_pool(name="big", bufs=1) as big, \
             tc.tile_pool(name="w_p", bufs=1) as w_p, \
             tc.tile_pool(name="sm2", bufs=3) as sm2, \
             tc.tile_pool(name="att_p", bufs=6) as att_p, \
             tc.tile_pool(name="y_p", bufs=6) as y_p, \
             tc.tile_pool(name="ps_acc", bufs=2, space="PSUM") as ps_acc, \
             tc.tile_pool(name="ps_st", bufs=2, space="PSUM") as ps_st, \
             tc.tile_pool(name="ps_av", bufs=2, space="PSUM") as ps_av:

            # ---- persistent SBUF tensors ----
            wq_sb = w_p.tile([128, HC, 128], BF16, tag="wq")
            wk_sb = w_p.tile([128, HC, 128], BF16, tag="wk")
            wv_sb = w_p.tile([128, HC, 128], BF16, tag="wv")
            wo_sb = w_p.tile([128, HIDDEN], BF16, tag="wo")
            tri_sb = w_p.tile([128, 128], BF16, tag="tri")
            pmat_sb = w_p.tile([128, 128], BF16, tag="pmat")
            bsel_sb = w_p.tile([2, 128], BF16, tag="bsel")
            ident = w_p.tile([128, 128], BF16, tag="ident")
            cs_sb = w_p.tile([128, L], BF16, tag="cs")
            sn_sb = w_p.tile([128, L], BF16, tag="sn")
            xt_sb = w_p.tile([128, HC, L], BF16, tag="xt")

            qro = [big.tile([128, 512], BF16, tag=f"qro{n}", name=f"qro{n}")
                   for n in range(LC)]
            kro = [big.tile([128, 512], BF16, tag=f"kro{n}", name=f"kro{n}")
                   for n in range(LC)]
            outT = [big.tile([128, 512], BF16, tag=f"out{n}", name=f"out{n}")
                    for n in range(LC)]
            vno = [big.tile([128, 130], BF16, tag=f"vno{k}", name=f"vno{k}")
                   for k in range(KVC)]

            # ---- prologue ----
            # HAM warmup: the PE clock-gate needs ~3.4us of sustained
            # activity to unthrottle 1.2->2.4GHz; run dummy matmuls on a
            # memset tile while the first weight/x DMAs are in flight
            dmy = w_p.tile([128, 256], BF16, tag="dmy")
            nc.gpsimd.memset(dmy, 1.0)
            for _ in range(12):
                dps = ps_acc.tile([128, 256], F32, tag="acc")
                nc.tensor.matmul(dps, dmy[:, 0:128], dmy)
            # critical DMA issues spread over 3 engine
            # queues (descriptor issue is ~600ns serial per engine) ----
            for quad in range(4):
                ks = slice(quad * 2 * 128, (quad + 1) * 2 * 128)
                kt_ = slice(quad * 2, (quad + 1) * 2)
                nc.sync.dma_start(
                    out=wq_sb[:, kt_, :],
                    in_=wqT_d.ap()[ks, :].rearrange("(c p) m -> p c m", p=128))
                nc.sync.dma_start(
                    out=xt_sb[:, kt_, 0:512],
                    in_=xT_d.ap()[ks, 0:512]
                        .rearrange("(c p) m -> p c m", p=128))
                nc.gpsimd.dma_start(
                    out=wk_sb[:, kt_, :],
                    in_=wkT_d.ap()[ks, :].rearrange("(c p) m -> p c m", p=128))
                nc.scalar.dma_start(
                    out=wv_sb[:, kt_, :],
                    in_=wvT_d.ap()[ks, :].rearrange("(c p) m -> p c m", p=128))
            nc.scalar.dma_start(out=cs_sb[:, 0:512], in_=cosT_d.ap()[:, 0:512])
            nc.scalar.dma_start(out=sn_sb[:, 0:512], in_=sinT_d.ap()[:, 0:512])
            nc.scalar.dma_start(out=pmat_sb, in_=pmat_d.ap())
            nc.scalar.dma_start(out=tri_sb, in_=tri_d.ap())
            nc.scalar.dma_start(out=bsel_sb, in_=bsel_d.ap())
            # staggered bulk prefetch: chunk n's x (4-way k-split) + rope
            # tables, issued from the gpsimd queue
            def prefetch(n):
                if n >= LC:
                    return
                ns = slice(n * 512, (n + 1) * 512)
                for kh in range(4):
                    nc.gpsimd.dma_start(
                        out=xt_sb[:, 2 * kh:2 * kh + 2, ns],
                        in_=xT_d.ap()[kh * 256:(kh + 1) * 256, ns]
                            .rearrange("(c p) m -> p c m", p=128))
                nc.gpsimd.dma_start(out=cs_sb[:, ns], in_=cosT_d.ap()[:, ns])
                nc.gpsimd.dma_start(out=sn_sb[:, ns], in_=sinT_d.ap()[:, ns])

            # prefetch(1) first so chunk 1's x lands before phase1(1);
            # then ones columns of v tiles (vno[kc] needed from
            # attention(kc//4) onward)
            prefetch(1)
            for kc in range(KVC):
                nc.gpsimd.memset(
                    vno[kc].rearrange("p (g c) -> p g c", c=65)[:, :, 64:65],
                    1.0)
            make_identity(nc, ident)

            # ---- phase1: projections + rope + v transpose for chunk n ----
            def accum(n, w_sb, dst_tag):
                ns = slice(n * 512, (n + 1) * 512)
                ps = ps_acc.tile([128, 512], F32, tag="acc")
                for k in range(HC):
                    nc.tensor.matmul(ps, w_sb[:, k, :], xt_sb[:, k, ns],
                                     start=(k == 0), stop=(k == HC - 1))
                t = sm2.tile([128, 512], BF16, tag=dst_tag)
                nc.vector.tensor_copy(t, ps)
                return ps, t

            def rope(n, t_sb, ro):
                ns = slice(n * 512, (n + 1) * 512)
                sw = ps_acc.tile([128, 512], F32, tag="acc")
                nc.tensor.matmul(sw, pmat_sb, t_sb)
                t1 = sm2.tile([128, 512], BF16, tag="t1")
                t2 = sm2.tile([128, 512], BF16, tag="t2")
                nc.vector.tensor_mul(t1, t_sb, cs_sb[:, ns])
                nc.vector.tensor_mul(t2, sw, sn_sb[:, ns])
                nc.vector.tensor_add(ro, t1, t2)

            def p1_parts(n):
                if n >= LC:
                    return []

                def qpart():
                    _, qt = accum(n, wq_sb, "qt")
                    rope(n, qt, qro[n])

                def kpart():
                    _, kt = accum(n, wk_sb, "kt")
                    rope(n, kt, kro[n])

                def vpart():
                    if n == 0:  # wo needed from the first wo projection on
                        nc.gpsimd.dma_start(out=wo_sb[:, 0:512],
                                            in_=woT_d.ap()[:, 0:512])
                        nc.gpsimd.dma_start(out=wo_sb[:, 512:1024],
                                            in_=woT_d.ap()[:, 512:1024])
                    prefetch(n + 2)
                    _, vt = accum(n, wv_sb, "vt")
                    for j in range(4):
                        kc = n * 4 + j
                        tr = ps_acc.tile([128, 128], BF16, tag="acc")
                        nc.tensor.transpose(tr, vt[:, j * 128:(j + 1) * 128],
                                            ident)
                        nc.vector.tensor_copy(
                            vno[kc]
                                .rearrange("p (g c) -> p g c", c=65)[:, :, 0:64],
                            tr.rearrange("p (g c) -> p g c", c=64))

                return [qpart, kpart, vpart]

            # ---- attention for q chunk qc (list of emission steps) ----
            def att_steps(qc):
                qs = slice(qc * 512, (qc + 1) * 512)
                n_kc = 4 * (qc + 1)
                n_g = n_kc // 2
                state = {}

                def qlo_of(kc):
                    d = kc - 4 * qc
                    return 128 * d if d >= 0 else 0

                def st_exp(h, g):
                    hs = h * 64
                    stp = ps_st.tile([128, 2, 512], F32, tag="st")
                    att = att_p.tile([128, 2, 512], BF16, tag="att")
                    for j in range(2):
                        kc = 2 * g + j
                        qlo = qlo_of(kc)
                        nc.tensor.matmul(
                            stp[:, j, qlo:],
                            kro[kc // 4][hs:hs + 64,
                                         (kc % 4) * 128:(kc % 4 + 1) * 128],
                            qro[qc][hs:hs + 64, qlo:])
                    if 2 * g >= 4 * qc:  # diagonal group: per-j trimmed exp
                        for j in range(2):
                            qlo = qlo_of(2 * g + j)
                            nc.scalar.activation(att[:, j, qlo:],
                                                 stp[:, j, qlo:], Exp)
                            nc.vector.tensor_mul(att[:, j, qlo:qlo + 128],
                                                 att[:, j, qlo:qlo + 128],
                                                 tri_sb)
                    else:
                        nc.scalar.activation(att, stp, Exp)
                    return att

                def av_mms(h, g_prev, att_prev):
                    for j in range(2):
                        kc = 2 * g_prev + j
                        qlo = qlo_of(kc)
                        nc.tensor.matmul(
                            state["avs"][h][:, qlo:],
                            vno[kc][:, h * 65:h * 65 + 65],
                            att_prev[:, j, qlo:],
                            start=(kc == 0), stop=(kc == n_kc - 1))

                steps = []

                def mk_gstep(g):
                    def gstep():
                        if g == 0:
                            state["avs"] = [ps_av.tile([65, 512], F32, tag="av",
                                                        name=f"av{qc}_{i}")
                                            for i in range(2)]
                            state["pending"] = [None, None]
                        for h in range(2):
                            att = st_exp(h, g)
                            if state["pending"][h] is not None:
                                av_mms(h, *state["pending"][h])
                            state["pending"][h] = (g, att)
                    return gstep

                for g in range(n_g):
                    steps.append(mk_gstep(g))

                def flush_norm():
                    avs = state["avs"]
                    # per-head: flush av, copy its denominator row, and
                    # start its pack DMA immediately (overlaps the other
                    # head's flush, so the reciprocal starts earlier)
                    den = sm2.tile([1, 1024], F32, tag="den")
                    dpk = sm2.tile([16, 64], F32, tag="dpk")
                    for h in range(2):
                        av_mms(h, *state["pending"][h])
                        nc.vector.tensor_copy(den[0:1, h * 512:(h + 1) * 512],
                                              avs[h][64:65, :])
                        nc.gpsimd.dma_start(
                            out=dpk[h * 8:(h + 1) * 8, :],
                            in_=den[0:1, h * 512:(h + 1) * 512]
                                .rearrange("o (p c) -> o p c", p=8))
                    # unnormalized attention output -> SBUF (releases the av
                    # PSUM slots now; deferred norm_b reads the SBUF copy)
                    av_sb = sm2.tile([128, 512], BF16, tag="avsb",
                                     name=f"avsb{qc}", bufs=2)
                    nc.vector.tensor_copy(av_sb[0:64, :], avs[0][0:64, :])
                    nc.vector.tensor_copy(av_sb[64:128, :], avs[1][0:64, :])
                    state["av_sb"] = av_sb
                    rpk = sm2.tile([16, 64], BF16, tag="rpk")
                    with nc.allow_low_precision(reason="bf16 1/denominator"):
                        nc.vector.reciprocal(rpk, dpk)
                    rden = sm2.tile([2, 512], BF16, tag="rden",
                                    name=f"rden{qc}", bufs=2)
                    nc.gpsimd.dma_start(
                        out=rden.rearrange("h (p c) -> h p c", p=8),
                        in_=rpk)
                    state["rden"] = rden

                def norm_b():
                    av_sb = state["av_sb"]
                    bc = ps_acc.tile([128, 512], F32, tag="acc")
                    nc.tensor.matmul(bc, bsel_sb, state["rden"])
                    bcs = sm2.tile([128, 512], F32, tag="bcs")
                    if qc == order[-1]:  # ACT is idle at the tail
                        nc.scalar.copy(bcs, bc)
                    else:
                        nc.vector.tensor_copy(bcs, bc)
                    nc.vector.tensor_mul(outT[qc], av_sb, bcs)
                steps.append(flush_norm)
                return steps, norm_b

            def wo_part(qc):
                qs = slice(qc * 512, (qc + 1) * 512)
                last = qc == LC - 1

                def run():
                    for e in range(HC):
                        ps_y = ps_acc.tile([128, 512], F32, tag="acc")
                        nc.tensor.matmul(ps_y, wo_sb[:, e * 128:(e + 1) * 128],
                                         outT[qc])
                        y_sb = y_p.tile([128, 512], BF16, tag="y")
                        if last and e % 2 == 0:
                            nc.scalar.copy(y_sb, ps_y)
                        else:
                            nc.vector.tensor_copy(y_sb, ps_y)
                        if last and e % 2 == 1:
                            nc.scalar.dma_start(
                                out=yT_d.ap()[e * 128:(e + 1) * 128, qs],
                                in_=y_sb)
                        else:
                            nc.sync.dma_start(
                                out=yT_d.ap()[e * 128:(e + 1) * 128, qs],
                                in_=y_sb)
                return run

            # ---- emission schedule ----
            # process attention chunks 1..7 then 0 (so the tiny chunk 0
            # forms the tail); phase1(qc) for the next chunk and the
            # previous chunk's norm+wo are interleaved into each attention
            for p in p1_parts(0):
                p()
            order = list(range(LC))
            norm_bs = {}
            for idx, qc in enumerate(order):
                steps, norm_b = att_steps(qc)
                norm_bs[qc] = norm_b
                extras = p1_parts(qc + 1)
                prev = order[idx - 1] if idx >= 1 else None
                merged = []
                ne, ns_ = len(extras), len(steps)
                wo_pos = max(1, (7 * ns_) // 10)
                ei = 0
                for i, s in enumerate(steps):
                    merged.append(s)
                    if prev is not None and i + 1 == wo_pos:
                        merged.append(norm_bs[prev])
                        merged.append(wo_part(prev))
                    want = (i + 1) * ne // ns_
                    while ei < want:
                        merged.append(extras[ei])
                        ei += 1
                while ei < ne:
                    merged.append(extras[ei])
                    ei += 1
                for f in merged:
                    f()
            norm_bs[order[-1]]()
            wo_part(order[-1])()

    nc.compile()
    return nc


def _host_prep(x, wq, wk, wv, wo, L):
    """Build per-core input maps (numpy only)."""
    import ml_dtypes
    BF = ml_dtypes.bfloat16

    x2 = np.ascontiguousarray(x.reshape(L, HIDDEN))
    xT = np.ascontiguousarray(x2.T.astype(BF))

    # rope tables, transposed + duplicated for the two heads on each core
    inv_freq = 1.0 / (ROPE_BASE ** (np.arange(0, HEAD_DIM, 2, dtype=np.float64)
                                    / HEAD_DIM))
    freqs = np.arange(L, dtype=np.float64)[:, None] * inv_freq[None, :]
    emb = np.concatenate([freqs, freqs], axis=-1)          # [L, 64]
    cosT = np.cos(emb).T
    sinT = np.sin(emb).T
    cosT2 = np.ascontiguousarray(np.concatenate([cosT, cosT], axis=0).astype(BF))
    sinT2 = np.ascontiguousarray(np.concatenate([sinT, sinT], axis=0).astype(BF))

    # triangular mask for the 128-wide diagonal kv blocks: keep q >= kv
    kv = np.arange(128)[:, None]
    q = np.arange(128)[None, :]
    tri = np.ascontiguousarray((q >= kv).astype(BF))

    # rotate-half permutation (as matmul lhsT), block-diag for 2 heads
    P = np.zeros((64, 64), np.float32)
    P[np.arange(32) + 32, np.arange(32)] = -1.0
    P[np.arange(32), np.arange(32) + 32] = 1.0
    pmat = np.zeros((128, 128), np.float32)
    pmat[0:64, 0:64] = P
    pmat[64:128, 64:128] = P
    pmat = np.ascontiguousarray(pmat.astype(BF))

    # denominator broadcast selector: row h -> partitions [64h, 64h+64)
    bsel = np.zeros((2, 128), np.float32)
    bsel[0, 0:64] = 1.0
    bsel[1, 64:128] = 1.0
    bsel = np.ascontiguousarray(bsel.astype(BF))

    in_maps = []
    for c in range(N_CORES):
        rows = slice(c * 128, (c + 1) * 128)
        in_maps.append({
            "xT": xT,
            "wqT": np.ascontiguousarray(
                (wq[rows, :].T * np.float32(1.0 / 8.0)).astype(BF)),
            "wkT": np.ascontiguousarray(wk[rows, :].T.astype(BF)),
            "wvT": np.ascontiguousarray(wv[rows, :].T.astype(BF)),
            "woT": np.ascontiguousarray(wo[:, rows].T.astype(BF)),
            "cosT": cosT2,
            "sinT": sinT2,
            "tri": tri,
            "pmat": pmat,
            "bsel": bsel,
        })
    return in_maps


def _ensure_profile_hook():
    """The agent image's antenv lacks axon_hooks; recreate it from the boot
    package so trace=True can capture NTFF profiles."""
    import sys, types
    try:
        from antenv.axon_hooks import get_axon_ntff_profile_hook  # noqa: F401
        return
    except ImportError:
        pass
    try:
        from trn_agent_boot.trn_boot import _ntff_profile_via_ctypes
        hook = _ntff_profile_via_ctypes('/opt/axon/libaxon_pjrt.so')
    except Exception:
        hook = None
    mod = types.ModuleType("antenv.axon_hooks")
    mod.get_axon_ntff_profile_hook = lambda: hook
    mod.set_axon_ntff_profile_hook = lambda h: None
    sys.modules["antenv.axon_hooks"] = mod


def _run(x, wq, wk, wv, wo, trace=False, trace_cores=None):
    from concourse.bass_utils import run_bass_kernel_spmd

    if trace:
        _ensure_profile_hook()

    B, L, D = x.shape
    assert (B, D) == (1, HIDDEN)
    if L not in _CACHE:
        _CACHE[L] = _build(L)
    nc = _CACHE[L]
    in_maps = _host_prep(np.asarray(x, np.float32), wq, wk, wv, wo, L)
    res = run_bass_kernel_spmd(
        nc, in_maps, core_ids=list(range(N_CORES)),
        trace=trace, trace_cores=trace_cores)
    acc = np.zeros((HIDDEN, L), np.float64)
    for r in res.results:
        acc += r["yT"].astype(np.float64)
    y = np.ascontiguousarray(acc.T.astype(np.float32)).reshape(1, L, HIDDEN)
    return y, res


def kernel(x, wq, wk, wv, wo):
    y, _ = _run(np.asarray(x), np.asarray(wq), np.asarray(wk),
                np.asarray(wv), np.asarray(wo))
    return y
